# revision 2
# baseline (speedup 1.0000x reference)
"""Trainium2 Bass kernel for nn_BalNoisedTopK (hinge loss with Monte-Carlo
smoothed top-(k+1) threshold).

reference:
    perturbed[b, j, :] = s[b, :] + eps * Z[b, :, j]
    kth[b, j]  = 6th largest of perturbed[b, j, :]     (k+1 = 6)
    skp1[b]    = mean_j kth[b, j]
    cs[b]      = s[b, y[b]]
    out        = mean_b relu(1 + skp1[b] - cs[b])

Sharding: data-parallel over batch B=1024 across 8 NeuronCores (128 rows per
core = the SBUF partition dim). Inside each core (mode "planar", the shipping
config):

  1. DMA streams s/Z d-chunks into SBUF (HWDGE, ~5 MB per chunk, the ~300 us
     HBM roofline for the 98 MB/core).
  2. The otherwise-idle ScalarEngine rearranges each (d, j)-interleaved chunk
     into j-planar layout with one strided-read/contiguous-write Copy per
     chunk. (The DVE top-8 op runs at half rate on strided input, so paying
     the rearrange on ACT keeps the critical DVE path at full rate.)
  3. The adds pert = Z + s (s broadcast over the noise axis via a 0-step AP)
     run dense on contiguous planes, split DVE (planes 0-2) / GPSIMD (3-4).
  4. The DVE InstMax op (top-8 per partition per instruction) reduces each
     (chunk, j) plane to 8 candidates; the union of per-chunk top-8s provably
     contains each row's global top-6 (any top-6 element has at most 5 larger
     elements anywhere, so it is within its own chunk's top-6), so a final
     InstMax over the candidate list yields the exact 6th-largest, ties and
     duplicate multiplicity included.
  5. correct_scores = s[b, y[b]] is a single indirect DMA row-gather using
     host-precomputed flat indices b*D + y[b].
  6. hinge = relu(1 + mean_j kth - cs) is computed on-chip; the host gathers
     the 8x[128] hinge vectors and takes the mean.

Shipping mode "planar4s" refines step 2-3: ACT rearranges only planes 0-3
(one strided-read Copy per chunk); plane 4 is never rearranged - it gets a
strided in-place GPSIMD add and a strided DVE InstMax directly on the
interleaved chunk, cutting the plane-4 rearrange out of the total work.
Adds: DVE planes 0-1, GPSIMD planes 2-3 (dense) + plane 4 (strided).

Measured on HW (8 cores in parallel): ~381 us/core steady-state throughput
(per-iteration marginal in a repeat loop; consecutive iterations overlap via
the continuously-streaming DMA rings) vs a ~302 us DMA-only floor for the
same loop structure; a fully serialized body (back-to-back in one program,
including pipeline fill+drain) measures ~780 us (planar). Bit-exact against
the jax reference (relative error 0.0).
"""

import sys

for _p in ("/opt/trn_rl_repo",):
    if _p not in sys.path:
        sys.path.insert(0, _p)

import numpy as np

B, D, NS = 1024, 32000, 5
K = 5          # top-(K+1); kth index = K (0-based) in descending order
EPS = 1.0      # noise scale (folded into the add since EPS == 1.0)
NCORES = 8
BSH = B // NCORES   # 128 rows per core = partition dim

DCH = 1600          # d-columns per streamed chunk
NCHUNK = D // DCH


_cache = {}


def _build(reps=1, mode="full", dch=None, zbufs=3, pbufs=2, nbody=1):
    global DCH, NCHUNK
    if dch is not None:
        DCH, NCHUNK = dch, D // dch
    import contextlib

    import concourse.bacc as bacc
    import concourse.mybir as mybir
    import concourse.tile as tile

    f32 = mybir.dt.float32
    nc = bacc.Bacc("TRN2", debug=False)
    s = nc.dram_tensor("s", [BSH, D], f32, kind="ExternalInput").ap()
    z = nc.dram_tensor("z", [BSH, D * NS], f32, kind="ExternalInput").ap()
    yv = nc.dram_tensor("yv", [BSH, 1], f32, kind="ExternalInput").ap()
    yi = nc.dram_tensor("yi", [BSH, 1], mybir.dt.int32, kind="ExternalInput").ap()
    out = nc.dram_tensor("hinge", [BSH, 1], f32, kind="ExternalOutput").ap()

    with tile.TileContext(nc) as tc:
        with (
            tc.tile_pool(name="zp", bufs=zbufs) as zp,
            tc.tile_pool(name="pp", bufs=pbufs) as pp,
            tc.tile_pool(name="sp", bufs=3) as sp,
            tc.tile_pool(name="scr", bufs=2) as scrp,
            tc.tile_pool(name="small", bufs=1) as smp,
        ):
            iota = smp.tile([BSH, DCH], f32)
            nc.gpsimd.iota(
                iota[:, :],
                pattern=[[1, DCH]],
                base=0,
                channel_multiplier=0,
                allow_small_or_imprecise_dtypes=True,
            )
            yv_t = smp.tile([BSH, 1], f32)
            nc.sync.dma_start(yv_t[:, :], yv)

            loop = tc.For_i(0, reps, 1) if reps > 1 else contextlib.nullcontext()
            with loop:
                for _nb in range(nbody):
                    _emit_body(nc, tc, zp, pp, sp, scrp, smp, s, z, yi, out, yv_t, iota, mode)

    nc.compile()
    return nc


def _emit_body(nc, tc, zp, pp, sp, scrp, smp, s, z, yi, out, yv_t, iota, mode="full"):
    import concourse.mybir as mybir

    f32 = mybir.dt.float32
    if True:
        if True:
            nseg = NCHUNK * 2 if mode == "planar2h" else NCHUNK
            cand = smp.tile([BSH, NS * nseg * 8], f32, tag="cand")
            csp = smp.tile([BSH, NCHUNK], f32, tag="csp")

            if mode != "dmaonly":
                import concourse.bass as bass

                ioff = smp.tile([BSH, 1], mybir.dt.int32, tag="ioff")
                nc.sync.dma_start(ioff[:, :], yi)
                cs_t = smp.tile([BSH, 1], f32, tag="cs_t")
                s_flat = s.rearrange("p d -> (p d)").unsqueeze(-1)
                nc.gpsimd.indirect_dma_start(
                    out=cs_t[:, :],
                    out_offset=None,
                    in_=s_flat,
                    in_offset=bass.IndirectOffsetOnAxis(ap=ioff[:, :1], axis=0),
                )

            if mode in ("bfp", "bfp3", "bfp4", "bfi", "bfi3"):
                # bf16 compute path. pert values are rounded to bf16; the
                # end-to-end loss error measured on the actual inputs is
                # 4e-5 relative (gate: 2e-2).
                bf = mybir.dt.bfloat16
                candb = smp.tile([BSH, NS * nseg * 8], bf, tag="candb")
                for i in range(NCHUNK):
                    st = sp.tile([BSH, DCH], f32, tag="st")
                    stb = sp.tile([BSH, DCH], bf, tag="stb")
                    if mode in ("bfi", "bfi3"):
                        # SWDGE cast-DMA: f32 HBM -> bf16 SBUF, stays
                        # (d, j)-interleaved; adds+max run strided.
                        ztb = zp.tile([BSH, DCH * NS], bf, tag="ztb")
                        nc.gpsimd.dma_start(
                            ztb[:, :], z[:, i * DCH * NS : (i + 1) * DCH * NS]
                        )
                        nc.sync.dma_start(st[:, :], s[:, i * DCH : (i + 1) * DCH])
                        nc.scalar.activation(
                            stb[:, :], st[:, :], mybir.ActivationFunctionType.Copy
                        )
                        ztv = ztb[:, :].rearrange("p (d j) -> p d j", j=NS)
                        if mode == "bfi3":
                            dsp = (DCH * 3) // 5
                            sb0 = (
                                stb[:, :dsp]
                                .unsqueeze(-1)
                                .to_broadcast([BSH, dsp, NS])
                            )
                            nc.vector.tensor_add(
                                ztv[:, :dsp, :], ztv[:, :dsp, :], sb0
                            )
                            sb1 = (
                                stb[:, dsp:]
                                .unsqueeze(-1)
                                .to_broadcast([BSH, DCH - dsp, NS])
                            )
                            nc.gpsimd.tensor_add(
                                ztv[:, dsp:, :], ztv[:, dsp:, :], sb1
                            )
                        else:
                            sb0 = stb[:, :].unsqueeze(-1).to_broadcast(
                                [BSH, DCH, NS]
                            )
                            nc.vector.tensor_add(ztv, ztv, sb0)
                        ztj = ztb[:, :].rearrange("p (d j) -> p j d", j=NS)
                        for j in range(NS):
                            o = (j * NCHUNK + i) * 8
                            nc.vector.max(out=candb[:, o : o + 8], in_=ztj[:, j, :])
                    else:
                        # HWDGE f32 DMA; ACT planarizes+casts to bf16; DVE
                        # adds ndve planes (GPS the rest); DVE InstMax dense.
                        ndve = {"bfp3": 3, "bfp4": 4}.get(mode, 5)
                        zt = zp.tile([BSH, DCH * NS], f32, tag="zt")
                        nc.sync.dma_start(
                            zt[:, :], z[:, i * DCH * NS : (i + 1) * DCH * NS]
                        )
                        nc.sync.dma_start(st[:, :], s[:, i * DCH : (i + 1) * DCH])
                        pt = pp.tile([BSH, NS * DCH], bf, tag="pt")
                        src_v = zt[:, :].rearrange("p (d j) -> p j d", j=NS)
                        dst_v = pt[:, :].rearrange("p (j d) -> p j d", j=NS)
                        nc.scalar.activation(
                            dst_v, src_v, mybir.ActivationFunctionType.Copy
                        )
                        nc.scalar.activation(
                            stb[:, :], st[:, :], mybir.ActivationFunctionType.Copy
                        )
                        sba = (
                            stb[:, :]
                            .unsqueeze(-1)
                            .rearrange("p d one -> p one d")
                            .to_broadcast([BSH, ndve, DCH])
                        )
                        va = pt[:, : ndve * DCH].rearrange(
                            "p (j d) -> p j d", j=ndve
                        )
                        nc.vector.tensor_add(va, va, sba)
                        if ndve < NS:
                            sbb = (
                                stb[:, :]
                                .unsqueeze(-1)
                                .rearrange("p d one -> p one d")
                                .to_broadcast([BSH, NS - ndve, DCH])
                            )
                            vb = pt[:, ndve * DCH :].rearrange(
                                "p (j d) -> p j d", j=NS - ndve
                            )
                            nc.gpsimd.tensor_add(vb, vb, sbb)
                        for j in range(NS):
                            o = (j * NCHUNK + i) * 8
                            nc.vector.max(
                                out=candb[:, o : o + 8],
                                in_=pt[:, j * DCH : (j + 1) * DCH],
                            )

                kth = smp.tile([BSH, NS], f32)
                for j in range(NS):
                    t8b = scrp.tile([BSH, 8], bf, tag="t8b")
                    nc.vector.max(
                        out=t8b[:, :],
                        in_=candb[:, j * nseg * 8 : (j + 1) * nseg * 8],
                    )
                    nc.vector.tensor_copy(kth[:, j : j + 1], t8b[:, K : K + 1])
                skp1 = smp.tile([BSH, 1], f32)
                nc.vector.tensor_reduce(
                    out=skp1[:, :],
                    in_=kth[:, :],
                    op=mybir.AluOpType.add,
                    axis=mybir.AxisListType.X,
                )
                h = smp.tile([BSH, 1], f32)
                nc.vector.tensor_scalar_mul(h[:, :], skp1[:, :], 1.0 / NS)
                nc.vector.tensor_sub(h[:, :], h[:, :], cs_t[:, :])
                nc.vector.tensor_scalar_add(h[:, :], h[:, :], 1.0)
                nc.vector.tensor_scalar_max(h[:, :], h[:, :], 0.0)
                nc.sync.dma_start(out, h[:, :])
                return

            if mode in ("planarR", "planarR23", "planarR05"):
                sizes = [500, 1500] + [2000] * 14 + [1500, 500]
                assert sum(sizes) == D
                ndve = {"planarR23": 2, "planarR05": 0}.get(mode, 3)
                nseg = len(sizes)
                cand = smp.tile([BSH, NS * nseg * 8], f32, tag="cand")
                off = 0
                for i, sz in enumerate(sizes):
                    zt = zp.tile([BSH, DCH * NS], f32, tag="zt")
                    st = sp.tile([BSH, DCH], f32, tag="st")
                    nc.sync.dma_start(
                        zt[:, : sz * NS], z[:, off * NS : (off + sz) * NS]
                    )
                    nc.sync.dma_start(st[:, :sz], s[:, off : off + sz])
                    pt = pp.tile([BSH, NS * DCH], f32, tag="pt")
                    src_v = zt[:, : sz * NS].rearrange("p (d j) -> p j d", j=NS)
                    dst_v = pt[:, : sz * NS].rearrange("p (j d) -> p j d", j=NS)
                    nc.scalar.activation(
                        dst_v, src_v, mybir.ActivationFunctionType.Copy
                    )
                    if ndve > 0:
                        sbA = (
                            st[:, :sz]
                            .unsqueeze(-1)
                            .rearrange("p d one -> p one d")
                            .to_broadcast([BSH, ndve, sz])
                        )
                        vA = pt[:, : ndve * sz].rearrange(
                            "p (j d) -> p j d", j=ndve
                        )
                        nc.vector.tensor_add(vA, vA, sbA)
                    sbB = (
                        st[:, :sz]
                        .unsqueeze(-1)
                        .rearrange("p d one -> p one d")
                        .to_broadcast([BSH, NS - ndve, sz])
                    )
                    vB = pt[:, ndve * sz : NS * sz].rearrange(
                        "p (j d) -> p j d", j=NS - ndve
                    )
                    nc.gpsimd.tensor_add(vB, vB, sbB)
                    for j in range(NS):
                        o = (j * nseg + i) * 8
                        nc.vector.max(
                            out=cand[:, o : o + 8],
                            in_=pt[:, j * sz : (j + 1) * sz],
                        )
                    off += sz
            else:
              for i in range(NCHUNK):
                zt = zp.tile([BSH, DCH * NS], f32, tag="zt")
                st = sp.tile([BSH, DCH], f32, tag="st")
                nc.sync.dma_start(zt[:, :], z[:, i * DCH * NS : (i + 1) * DCH * NS])
                nc.sync.dma_start(st[:, :], s[:, i * DCH : (i + 1) * DCH])

                # pert = Z + s  (broadcast s over the inner noise axis), in place
                if mode in ("planar4s", "planar4s1"):
                    # ACT rearranges only planes 0-3; plane 4 stays interleaved
                    # in zt (strided GPSIMD add + strided InstMax) - cuts the
                    # plane-4 rearrange out of the total work entirely.
                    ndve = 1 if mode == "planar4s1" else 2
                    pt = pp.tile([BSH, 4 * DCH], f32, tag="pt")
                    src_v = zt[:, :].rearrange("p (d j) -> p j d", j=NS)
                    dst_v = pt[:, :].rearrange("p (j d) -> p j d", j=4)
                    nc.scalar.activation(
                        dst_v, src_v[:, :4, :], mybir.ActivationFunctionType.Copy
                    )
                    sba = (
                        st[:, :]
                        .unsqueeze(-1)
                        .rearrange("p d one -> p one d")
                        .to_broadcast([BSH, ndve, DCH])
                    )
                    va = pt[:, : ndve * DCH].rearrange("p (j d) -> p j d", j=ndve)
                    nc.vector.tensor_add(va, va, sba)
                    sbb = (
                        st[:, :]
                        .unsqueeze(-1)
                        .rearrange("p d one -> p one d")
                        .to_broadcast([BSH, 4 - ndve, DCH])
                    )
                    vb = pt[:, ndve * DCH :].rearrange(
                        "p (j d) -> p j d", j=4 - ndve
                    )
                    nc.gpsimd.tensor_add(vb, vb, sbb)
                    z4 = src_v[:, 4, :]
                    nc.gpsimd.tensor_add(z4, z4, st[:, :])
                    for j in range(4):
                        o = (j * NCHUNK + i) * 8
                        nc.vector.max(
                            out=cand[:, o : o + 8],
                            in_=pt[:, j * DCH : (j + 1) * DCH],
                        )
                    o = (4 * NCHUNK + i) * 8
                    nc.vector.max(out=cand[:, o : o + 8], in_=z4)
                elif mode == "planarS":
                    # split planar tiles: pa (planes 0-2, ACT->DVE add->max),
                    # pb (planes 3-4, ACT->GPS add->max) rotate independently
                    pa = pp.tile([BSH, 3 * DCH], f32, tag="pa")
                    pb = pp.tile([BSH, 2 * DCH], f32, tag="pb")
                    src_v = zt[:, :].rearrange("p (d j) -> p j d", j=NS)
                    da = pa[:, :].rearrange("p (j d) -> p j d", j=3)
                    db = pb[:, :].rearrange("p (j d) -> p j d", j=2)
                    nc.scalar.activation(
                        da, src_v[:, :3, :], mybir.ActivationFunctionType.Copy
                    )
                    nc.scalar.activation(
                        db, src_v[:, 3:, :], mybir.ActivationFunctionType.Copy
                    )
                    sb3 = (
                        st[:, :]
                        .unsqueeze(-1)
                        .rearrange("p d one -> p one d")
                        .to_broadcast([BSH, 3, DCH])
                    )
                    nc.vector.tensor_add(da, da, sb3)
                    sb2 = (
                        st[:, :]
                        .unsqueeze(-1)
                        .rearrange("p d one -> p one d")
                        .to_broadcast([BSH, 2, DCH])
                    )
                    nc.gpsimd.tensor_add(db, db, sb2)
                    for j in range(NS):
                        o = (j * NCHUNK + i) * 8
                        srcm = (
                            pa[:, j * DCH : (j + 1) * DCH]
                            if j < 3
                            else pb[:, (j - 3) * DCH : (j - 2) * DCH]
                        )
                        nc.vector.max(out=cand[:, o : o + 8], in_=srcm)
                elif mode in ("planarI", "planarI4"):
                    # adds FIRST on the interleaved chunk (d-contiguous split
                    # DVE/GPSIMD), then rearrange the sum to j-planar
                    # (ACT 4 or 5 planes, GPSIMD 1), then contiguous InstMax.
                    dsp = (DCH * 12) // 25
                    ztv = zt[:, :].rearrange("p (d j) -> p d j", j=NS)
                    sb0 = st[:, :dsp].unsqueeze(-1).to_broadcast([BSH, dsp, NS])
                    nc.vector.tensor_add(ztv[:, :dsp, :], ztv[:, :dsp, :], sb0)
                    sb1 = st[:, dsp:].unsqueeze(-1).to_broadcast(
                        [BSH, DCH - dsp, NS]
                    )
                    nc.gpsimd.tensor_add(ztv[:, dsp:, :], ztv[:, dsp:, :], sb1)
                    pt = pp.tile([BSH, NS * DCH], f32, tag="pt")
                    src_v = zt[:, :].rearrange("p (d j) -> p j d", j=NS)
                    dst_v = pt[:, :].rearrange("p (j d) -> p j d", j=NS)
                    if mode == "planarI4":
                        nc.scalar.activation(
                            dst_v[:, :4, :],
                            src_v[:, :4, :],
                            mybir.ActivationFunctionType.Copy,
                        )
                        nc.gpsimd.tensor_copy(dst_v[:, 4, :], src_v[:, 4, :])
                    else:
                        nc.scalar.activation(
                            dst_v, src_v, mybir.ActivationFunctionType.Copy
                        )
                elif mode == "planar2h":
                    # half-d compute granularity over one DMA chunk
                    H = DCH // 2
                    for h in range(2):
                        pt = pp.tile([BSH, NS * H], f32, tag=f"pt{h}")
                        src_v = zt[:, :].rearrange("p (d j) -> p j d", j=NS)[
                            :, :, h * H : (h + 1) * H
                        ]
                        dst_v = pt[:, :].rearrange("p (j d) -> p j d", j=NS)
                        nc.scalar.activation(
                            dst_v, src_v, mybir.ActivationFunctionType.Copy
                        )
                        sth = st[:, h * H : (h + 1) * H]
                        sb3 = (
                            sth.unsqueeze(-1)
                            .rearrange("p d one -> p one d")
                            .to_broadcast([BSH, 3, H])
                        )
                        v3 = pt[:, : 3 * H].rearrange("p (j d) -> p j d", j=3)
                        nc.vector.tensor_add(v3, v3, sb3)
                        sb2 = (
                            sth.unsqueeze(-1)
                            .rearrange("p d one -> p one d")
                            .to_broadcast([BSH, 2, H])
                        )
                        v2 = pt[:, 3 * H :].rearrange("p (j d) -> p j d", j=2)
                        nc.gpsimd.tensor_add(v2, v2, sb2)
                        for j in range(NS):
                            o = (j * NCHUNK * 2 + i * 2 + h) * 8
                            nc.vector.max(
                                out=cand[:, o : o + 8],
                                in_=pt[:, j * H : (j + 1) * H],
                            )
                elif mode == "planar4":
                    # ACT rearranges planes 0-3, GPSIMD rearranges plane 4
                    pt = pp.tile([BSH, NS * DCH], f32, tag="pt")
                    src_v = zt[:, :].rearrange("p (d j) -> p j d", j=NS)
                    dst_v = pt[:, :].rearrange("p (j d) -> p j d", j=NS)
                    nc.scalar.activation(
                        dst_v[:, :4, :],
                        src_v[:, :4, :],
                        mybir.ActivationFunctionType.Copy,
                    )
                    nc.gpsimd.tensor_copy(dst_v[:, 4, :], src_v[:, 4, :])
                    sb3 = (
                        st[:, :]
                        .unsqueeze(-1)
                        .rearrange("p d one -> p one d")
                        .to_broadcast([BSH, 3, DCH])
                    )
                    v3 = pt[:, : 3 * DCH].rearrange("p (j d) -> p j d", j=3)
                    nc.vector.tensor_add(v3, v3, sb3)
                    sb2 = (
                        st[:, :]
                        .unsqueeze(-1)
                        .rearrange("p d one -> p one d")
                        .to_broadcast([BSH, 2, DCH])
                    )
                    v2 = pt[:, 3 * DCH :].rearrange("p (j d) -> p j d", j=2)
                    nc.gpsimd.tensor_add(v2, v2, sb2)
                elif mode == "planar":
                    # 1) ACT rearranges the interleaved chunk to j-planar
                    #    (strided read, contiguous write), one op per chunk
                    pt = pp.tile([BSH, NS * DCH], f32, tag="pt")
                    src_v = zt[:, :].rearrange("p (d j) -> p j d", j=NS)
                    dst_v = pt[:, :].rearrange("p (j d) -> p j d", j=NS)
                    nc.scalar.activation(
                        dst_v, src_v, mybir.ActivationFunctionType.Copy
                    )
                    # 2) dense adds on contiguous planes: DVE planes 0-2,
                    #    GPSIMD planes 3-4
                    sb3 = (
                        st[:, :]
                        .unsqueeze(-1)
                        .rearrange("p d one -> p one d")
                        .to_broadcast([BSH, 3, DCH])
                    )
                    v3 = pt[:, : 3 * DCH].rearrange("p (j d) -> p j d", j=3)
                    nc.vector.tensor_add(v3, v3, sb3)
                    sb2 = (
                        st[:, :]
                        .unsqueeze(-1)
                        .rearrange("p d one -> p one d")
                        .to_broadcast([BSH, 2, DCH])
                    )
                    v2 = pt[:, 3 * DCH :].rearrange("p (j d) -> p j d", j=2)
                    nc.gpsimd.tensor_add(v2, v2, sb2)
                elif mode == "split":
                    # d-contiguous split of the add between DVE and GPSIMD
                    dsp = (DCH * 9) // 20
                    ztv = zt[:, :].rearrange("p (d j) -> p d j", j=NS)
                    sb0 = st[:, :dsp].unsqueeze(-1).to_broadcast([BSH, dsp, NS])
                    nc.vector.tensor_add(ztv[:, :dsp, :], ztv[:, :dsp, :], sb0)
                    sb1 = st[:, dsp:].unsqueeze(-1).to_broadcast(
                        [BSH, DCH - dsp, NS]
                    )
                    nc.gpsimd.tensor_add(ztv[:, dsp:, :], ztv[:, dsp:, :], sb1)
                elif mode not in ("noadd", "dmaonly"):
                    ztv = zt[:, :].rearrange("p (d j) -> p d j", j=NS)
                    sb = st[:, :].unsqueeze(-1).to_broadcast([BSH, DCH, NS])
                    eng = nc.gpsimd if mode == "addgp" else nc.vector
                    eng.tensor_add(ztv, ztv, sb)

                # correct-score partial: sum_d (iota == (y - i*DCH)) * s_chunk
                if mode == "dmaonly":
                    # keep a data dependency on the tiles so DMA isn't dead-code
                    nc.vector.tensor_reduce(out=csp[:, i : i + 1], in_=zt[:, :8], op=mybir.AluOpType.add, axis=mybir.AxisListType.X)
                    nc.vector.tensor_reduce(out=cand[:, i : i + 1], in_=st[:, :8], op=mybir.AluOpType.add, axis=mybir.AxisListType.X)
                    continue

                # per-noise-sample top-8 of this chunk
                if mode in ("planar2h", "planarS", "planar4s", "planar4s1"):
                    pass
                elif mode in ("planar", "planar4", "planarI", "planarI4"):
                    for j in range(NS):
                        o = (j * NCHUNK + i) * 8
                        nc.vector.max(
                            out=cand[:, o : o + 8],
                            in_=pt[:, j * DCH : (j + 1) * DCH],
                        )
                elif mode != "nomax":
                    ztj = zt[:, :].rearrange("p (d j) -> p j d", j=NS)
                    for j in range(NS):
                        o = (j * NCHUNK + i) * 8
                        nc.vector.max(out=cand[:, o : o + 8], in_=ztj[:, j, :])

            # merge candidates per j, pick the (K+1)-th largest
            kth = smp.tile([BSH, NS], f32)
            if mode in ("nomax", "dmaonly"):
                for j in range(NS):
                    src_ap = csp[:, j : j + 1] if mode == "dmaonly" else cs_t[:, :1]
                    nc.vector.tensor_copy(kth[:, j : j + 1], src_ap)
            else:
                for j in range(NS):
                    t8 = scrp.tile([BSH, 8], f32, tag="t8")
                    nc.vector.max(
                        out=t8[:, :],
                        in_=cand[:, j * nseg * 8 : (j + 1) * nseg * 8],
                    )
                    nc.vector.tensor_copy(kth[:, j : j + 1], t8[:, K : K + 1])

            skp1 = smp.tile([BSH, 1], f32)
            nc.vector.tensor_reduce(
                out=skp1[:, :],
                in_=kth[:, :],
                op=mybir.AluOpType.add,
                axis=mybir.AxisListType.X,
            )
            if mode != "dmaonly":
                cs = cs_t
            else:
                cs = smp.tile([BSH, 1], f32)
                nc.vector.tensor_reduce(
                    out=cs[:, :],
                    in_=csp[:, :],
                    op=mybir.AluOpType.add,
                    axis=mybir.AxisListType.X,
                )

            # hinge = relu(1 + skp1/NS - cs)
            h = smp.tile([BSH, 1], f32)
            nc.vector.tensor_scalar_mul(h[:, :], skp1[:, :], 1.0 / NS)
            nc.vector.tensor_sub(h[:, :], h[:, :], cs[:, :])
            nc.vector.tensor_scalar_add(h[:, :], h[:, :], 1.0)
            nc.vector.tensor_scalar_max(h[:, :], h[:, :], 0.0)
            nc.sync.dma_start(out, h[:, :])


def _get_nc(reps=1, mode="full", dch=None, zbufs=3, pbufs=2, nbody=1):
    key = ("nc", reps, mode, dch, zbufs, pbufs, nbody)
    if key not in _cache:
        _cache[key] = _build(reps, mode, dch, zbufs, pbufs, nbody)
    return _cache[key]


def _make_in_maps(s, y, Z):
    s = np.asarray(s, dtype=np.float32)
    Z = np.asarray(Z, dtype=np.float32)
    y = np.asarray(y)
    in_maps = []
    for c in range(NCORES):
        rows = slice(c * BSH, (c + 1) * BSH)
        in_maps.append(
            {
                "s": np.ascontiguousarray(s[rows]),
                "z": np.ascontiguousarray(Z[rows].reshape(BSH, D * NS)),
                "yv": np.ascontiguousarray(
                    y[rows].astype(np.float32).reshape(BSH, 1)
                ),
                "yi": np.ascontiguousarray(
                    (np.arange(BSH, dtype=np.int64) * D + y[rows]).astype(
                        np.int32
                    ).reshape(BSH, 1)
                ),
            }
        )
    return in_maps


BEST = dict(mode="planar4s", dch=2000, zbufs=2, pbufs=2)


def _run(s, y, Z, trace=False):
    from concourse import bass_utils

    nc = _get_nc(1, BEST["mode"], BEST["dch"], BEST["zbufs"], BEST["pbufs"])
    in_maps = _make_in_maps(s, y, Z)
    res = bass_utils.run_bass_kernel_spmd(
        nc, in_maps, core_ids=list(range(NCORES)), trace=trace
    )
    hinges = np.concatenate(
        [res.results[c]["hinge"].reshape(-1) for c in range(NCORES)]
    )
    loss = np.float32(hinges.mean(dtype=np.float64))
    return loss, res


def kernel(s, y, Z):
    loss, _ = _run(s, y, Z, trace=False)
    return np.asarray(loss, dtype=np.float32)

